# revision 22
# baseline (speedup 1.0000x reference)
"""ArcFace (AngularPenaltySMLoss) on 8 TRN2 NeuronCores.

Sharding (model-parallel softmax): 32768 classes split across 8 cores
(4096 each). Host prep is layout/dtype only (transpose, gather, dtype
casts, index permutation). No host arithmetic.

v4 structure:
  - DMA: big-contiguous-row transfers split across both HWDGE queues
    (sync + scalar) in dependency-priority order; fnat/wtgt use a
    (p t) partition mapping (16KB contiguous per partition) and fT
    ships with columns permuted host-side to match (pure indexing:
    column tau*128+pi holds batch row pi*16+tau).
  - Weight-col norms group 0 upfront: DVE squares of full k-rows,
    PE matmuls with a ones[128,128] stationary leave the column sums
    replicated across partitions; ACT exp(-0.5*ln) -> rnr; per-(chunk,
    k-pair) DVE muls quantize to fp8 whats. Group 1's norm matmuls are
    HOSTED inside sweep-A tiles b8..b11 (each 512-chunk's rowsums live
    in that tile's PSUM before the mains start-clear it), so no PSUM
    slot is held across the sweep and the head stays short.
  - Main loop class-half-major, j-outer (stationary reuse), fp8
    DoubleRow; ACT Exp (scale=64/||f||) with accum_out row sums.
  - ssf/rawdot/wn2 on Pool (muls) + DVE (reduces), ssf first.
  - ONE AllGather of the locally pre-added sums at the very end:
    serial per-collective turnaround on the collective cores plus the
    cross-core pipeline offset made multiple AllGathers strictly
    worse; everything except the last denominator chain is
    precomputed during the sweep.
"""
import math

import numpy as np
import ml_dtypes

import concourse.bass as bass
import concourse.tile as tile
from concourse import bacc, mybir
from concourse.bass_utils import run_bass_kernel_spmd

B = 2048          # batch
D = 512           # feature dim
C = 32768         # classes
NCORES = 8
CS = C // NCORES  # 4096 classes per core
S = 64.0
MARGIN = 0.5
EPS = 1e-7
COSM = math.cos(MARGIN)
SINM = math.sin(MARGIN)

NB = B // 128     # 16 batch tiles
NK = D // 128     # 4 contraction chunks
NCC = CS // 512   # 8 class chunks per core

F32 = mybir.dt.float32
BF16 = mybir.dt.bfloat16
AF = mybir.ActivationFunctionType
ALU = mybir.AluOpType
BF16NP = ml_dtypes.bfloat16
FP8 = mybir.dt.float8e4
FP8NP = ml_dtypes.float8_e4m3fn
DR = mybir.MatmulPerfMode.DoubleRow

_CACHE = {}

_ONE_SET = "natural_log_exp_and_others"


def _patch_act_tables():
    from concourse import hw_specs, bacc as bacc_mod
    if getattr(bacc_mod, "_act_tables_patched", False):
        return
    orig = hw_specs.get_activation_tables

    def patched(arch):
        t = orig(arch)
        return {name: (funcs if name == _ONE_SET else set())
                for name, funcs in t.items()}

    bacc_mod.get_activation_tables = patched
    bacc_mod._act_tables_patched = True


def _build():
    _patch_act_tables()
    nc = bacc.Bacc(None, target_bir_lowering=False, debug=False)

    fT_ext = nc.declare_dram_parameter("fT", [D, B], FP8, isOutput=False)
    wT_ext = nc.declare_dram_parameter("wT", [D, CS], BF16, isOutput=False)
    fnat_ext = nc.declare_dram_parameter("fnat", [B, D], BF16, isOutput=False)
    wtgt_ext = nc.declare_dram_parameter("wtgt", [B, D], BF16, isOutput=False)
    out_ext = nc.declare_dram_parameter("out", [1, 1], F32, isOutput=True)

    ccT_in = nc.dram_tensor("ccT_in", [128, NB], F32)
    ccT_out = nc.dram_tensor("ccT_out", [128 * NCORES, NB], F32,
                             addr_space="Shared")
    ccW_in = nc.dram_tensor("ccW_in", [128, 1], F32)
    ccW_out = nc.dram_tensor("ccW_out", [128 * NCORES, 1], F32,
                             addr_space="Shared")

    with tile.TileContext(nc) as tc:
        with (
            tc.tile_pool(name="persist", bufs=1) as pp,
            tc.tile_pool(name="stream", bufs=4) as sp,
        ):
            # ---- persistent SBUF tiles ----
            wt3 = pp.tile([128, NK, CS], BF16)     # raw wT (bf16)
            whats = [pp.tile([128, NK, 512], FP8, tag=f"what{i}",
                             name=f"what{i}")
                     for i in range(NCC)]          # normalized wT, per chunk
            ft3 = pp.tile([128, NK, B], FP8)       # raw fT (fp8) = stationary
            fnat3 = pp.tile([128, NB, D], BF16)    # features, (p t) mapping
            wtgt3 = pp.tile([128, NB, D], BF16)    # target rows, (p t)
            ones128 = pp.tile([128, 128], BF16)
            ones_f32 = pp.tile([128, 1], F32)
            rnr_sb = pp.tile([128, NCC, 512], BF16)  # 1/||w_c||, replicated
            lnr_sb = pp.tile([128, 2048], F32)       # ln scratch
            ejunk = pp.tile([128, 2048], BF16)       # Exp write target
            sumsA = pp.tile([128, NB], F32)        # exp sums, chunks 0-3
            sumsB = pp.tile([128, NB], F32)        # exp sums, chunks 4-7
            rs_pt = pp.tile([128, NB], F32)        # 64/||f_b|| per-partition
            ssf = pp.tile([128, NB], F32)
            rawdot = pp.tile([128, NB], F32)
            wn2 = pp.tile([128, NB], F32)
            sq1 = [pp.tile([128, 2048], BF16, tag=f"sq{k}", name=f"sq{k}")
                   for k in range(NK)]             # squares of g1 k-rows

            # ---- DMAs: big rows, two hwdge queues, priority order ----
            wTr = wT_ext[:].rearrange("(k p) c -> p k c", p=128)
            fTr = fT_ext[:].rearrange("(k p) b -> p k b", p=128)
            fnr = fnat_ext[:].rearrange("(p t) d -> p t d", t=NB)
            wgr = wtgt_ext[:].rearrange("(p t) d -> p t d", t=NB)
            g0 = slice(0, 2048)
            g1 = slice(2048, 4096)
            nc.sync.dma_start(wt3[:, 0, g0], wTr[:, 0, g0])
            nc.scalar.dma_start(wt3[:, 2, g0], wTr[:, 2, g0])
            nc.sync.dma_start(wt3[:, 1, g0], wTr[:, 1, g0])
            nc.scalar.dma_start(wt3[:, 3, g0], wTr[:, 3, g0])
            nc.sync.dma_start(ft3[:], fTr[:])
            nc.scalar.dma_start(fnat3[:, 0:8, :], fnr[:, 0:8, :])
            nc.sync.dma_start(wt3[:, 0, g1], wTr[:, 0, g1])
            nc.scalar.dma_start(wt3[:, 2, g1], wTr[:, 2, g1])
            nc.sync.dma_start(wt3[:, 1, g1], wTr[:, 1, g1])
            nc.scalar.dma_start(wt3[:, 3, g1], wTr[:, 3, g1])
            nc.sync.dma_start(wtgt3[:], wgr[:])
            nc.scalar.dma_start(fnat3[:, 8:NB, :], fnr[:, 8:NB, :])

            nc.vector.memset(ones128[:], 1.0)
            nc.vector.memset(ones_f32[:], 1.0)

            # dummy AllGather: warms the collective mesh and absorbs the
            # cross-core launch skew in parallel with the compute, so the
            # real AllGather at the tail sees aligned peers and a formed
            # mesh. Nothing reads ccW_out.
            nc.sync.dma_start(ccW_in[:], ones_f32[:])
            nc.gpsimd.collective_compute(
                "AllGather", ALU.bypass,
                replica_groups=[list(range(NCORES))],
                ins=[ccW_in[:].opt()],
                outs=[ccW_out[:].opt()],
            )

            pmain_cm = tc.tile_pool(name="pmain", bufs=2, space="PSUM")
            pmain = pmain_cm.__enter__()

            # ---- group-0 weight-column norms (upfront) ----
            npsA = pmain.tile([128, 2048], F32, tag="z", name="npsA")
            for k in range(NK):
                sqk = sp.tile([128, 2048], BF16, tag="sqt", name="sqk")
                nc.vector.tensor_mul(sqk[:], wt3[:, k, g0], wt3[:, k, g0])
                for c4 in range(4):
                    nc.tensor.matmul(
                        npsA[:, bass.ts(c4, 512)],
                        ones128[:], sqk[:, bass.ts(c4, 512)],
                        start=(k == 0), stop=(k == NK - 1))
            for h in range(2):
                seg = npsA[:, h * 1024:(h + 1) * 1024]
                lseg = lnr_sb[:, h * 1024:(h + 1) * 1024]
                nc.scalar.activation(lseg, seg, AF.Ln)
                nc.scalar.activation(
                    rnr_sb[:, 2 * h:2 * h + 2, :]
                    .rearrange("p a b -> p (a b)"),
                    lseg, AF.Exp, scale=-0.5)

            # whats g0, j0 first (mains b0-j0 needs only the k0-k1 pair)
            def what_mul(cc, j):
                nc.vector.tensor_mul(
                    whats[cc][:, 2 * j:2 * j + 2, :],
                    wt3[:, 2 * j:2 * j + 2, bass.ts(cc, 512)],
                    rnr_sb[:, cc:cc + 1, :].broadcast_to([128, 2, 512]))

            for cc in range(4):
                what_mul(cc, 0)
            for cc in range(4):
                what_mul(cc, 1)
            # squares of the g1 k-rows: after whats-g0 (so mains are not
            # blocked) but before the Pool-paced reduces (so the hosted
            # sweep-A norm matmuls are never input-starved)
            for k in range(NK):
                nc.vector.tensor_mul(sq1[k][:], wt3[:, k, g1], wt3[:, k, g1])

            # ---- Pool muls + DVE reduces: ssf first ----
            for t in range(NB):
                sqf = sp.tile([128, D], BF16, tag="prod", name="sqf")
                nc.gpsimd.tensor_mul(sqf[:], fnat3[:, t, :], fnat3[:, t, :])
                nc.vector.reduce_sum(ssf[:, t:t + 1], sqf[:],
                                     axis=mybir.AxisListType.X)
            for t in range(NB):
                prod = sp.tile([128, D], BF16, tag="prod", name="prod")
                nc.gpsimd.tensor_mul(prod[:], fnat3[:, t, :], wtgt3[:, t, :])
                nc.vector.reduce_sum(rawdot[:, t:t + 1], prod[:],
                                     axis=mybir.AxisListType.X)
                sq2 = sp.tile([128, D], BF16, tag="prod", name="sq2")
                nc.gpsimd.tensor_mul(sq2[:], wtgt3[:, t, :], wtgt3[:, t, :])
                nc.vector.reduce_sum(wn2[:, t:t + 1], sq2[:],
                                     axis=mybir.AxisListType.X)

            # rs = 64/||f|| = exp(-0.5*ln(ssf/4096)); small first batches
            for h0, h1 in ((0, 2), (2, 4), (4, 8), (8, 12), (12, 16)):
                lcol = sp.tile([128, 4], F32, tag="lcol", name="lcol")
                nc.scalar.activation(lcol[:, 0:h1 - h0], ssf[:, h0:h1],
                                     AF.Ln, scale=1.0 / 4096.0)
                nc.scalar.activation(rs_pt[:, h0:h1], lcol[:, 0:h1 - h0],
                                     AF.Exp, scale=-0.5)

            # ---- main sweeps ----
            def mains(zp, g, b):
                for j in range(2):
                    for c4 in range(4):
                        cc = 4 * g + c4
                        nc.tensor.matmul(
                            zp[:, bass.ts(c4, 512)],
                            ft3[:, 2 * j:2 * j + 2, bass.ts(b, 128)],
                            whats[cc][:, 2 * j:2 * j + 2, :],
                            start=(j == 0), stop=(j == 1),
                            perf_mode=DR)

            # sweep A; tiles b8..b11 host the group-1 norm rowsums in
            # their PSUM pre-life (region is start-cleared by the mains
            # only after the Ln has read it)
            for b in range(NB):
                zp = pmain.tile([128, 2048], F32, tag="z", name="zp")
                if 8 <= b < 12:
                    cc = b - 4          # chunks 4..7
                    c4 = b - 8
                    for k in range(NK):
                        nc.tensor.matmul(
                            zp[:, 0:512], ones128[:],
                            sq1[k][:, bass.ts(c4, 512)],
                            start=(k == 0), stop=(k == NK - 1))
                    nc.scalar.activation(lnr_sb[:, 0:512], zp[:, 0:512],
                                         AF.Ln)
                    nc.scalar.activation(rnr_sb[:, cc, :], lnr_sb[:, 0:512],
                                         AF.Exp, scale=-0.5)
                    what_mul(cc, 0)
                    what_mul(cc, 1)
                mains(zp, 0, b)
                nc.scalar.activation(
                    ejunk[:], zp[:], AF.Exp, scale=rs_pt[:, b:b + 1],
                    accum_out=sumsA[:, b:b + 1])
            # sweep B
            for b in range(NB):
                zp = pmain.tile([128, 2048], F32, tag="z", name="zpB")
                mains(zp, 1, b)
                nc.scalar.activation(
                    ejunk[:], zp[:], AF.Exp, scale=rs_pt[:, b:b + 1],
                    accum_out=sumsB[:, b:b + 1])

            # ---- single AllGather of the pre-added sums ----
            sumsT = pp.tile([128, NB], F32)
            nc.vector.tensor_add(sumsT[:], sumsA[:], sumsB[:])
            nc.sync.dma_start(ccT_in[:], sumsT[:])
            nc.gpsimd.collective_compute(
                "AllGather", ALU.bypass,
                replica_groups=[list(range(NCORES))],
                ins=[ccT_in[:].opt()],
                outs=[ccT_out[:].opt()],
            )
            gathT = pp.tile([128, NCORES, NB], F32)
            nc.sync.dma_start(
                gathT[:], ccT_out[:].rearrange("(g p) c -> p g c", p=128))
            fullsum = pp.tile([128, NB], F32)
            nc.vector.tensor_reduce(
                fullsum[:], gathT[:].rearrange("p g c -> p c g"),
                axis=mybir.AxisListType.X, op=ALU.add)

            # ---- combine: everything except the denominator is
            # precomputed during the sweep ----
            m2 = pp.tile([128, NB], F32)
            nc.vector.tensor_mul(m2[:], ssf[:], wn2[:])
            lm2 = pp.tile([128, NB], F32)
            nc.scalar.activation(lm2[:], m2[:], AF.Ln)
            rboth = pp.tile([128, NB], F32)
            nc.scalar.activation(rboth[:], lm2[:], AF.Exp, scale=-0.5)
            tgt = pp.tile([128, NB], F32)
            nc.vector.tensor_mul(tgt[:], rawdot[:], rboth[:])
            exptgt = pp.tile([128, NB], F32)
            nc.scalar.activation(exptgt[:], tgt[:], AF.Exp, scale=S)
            tclip = pp.tile([128, NB], F32)
            nc.vector.tensor_scalar(
                tclip[:], tgt[:], -1.0 + EPS, 1.0 - EPS,
                op0=ALU.max, op1=ALU.min)
            om = pp.tile([128, NB], F32)
            nc.vector.tensor_mul(om[:], tclip[:], tclip[:])
            nc.vector.tensor_scalar(om[:], om[:], -1.0, 1.0,
                                    op0=ALU.mult, op1=ALU.add)
            lom = pp.tile([128, NB], F32)
            nc.scalar.activation(lom[:], om[:], AF.Ln)
            snt = pp.tile([128, NB], F32)
            nc.scalar.activation(snt[:], lom[:], AF.Exp, scale=0.5)
            num = pp.tile([128, NB], F32)
            nc.vector.tensor_scalar_mul(num[:], tclip[:], S * COSM)
            snts = pp.tile([128, NB], F32)
            nc.vector.tensor_scalar_mul(snts[:], snt[:], S * SINM)
            nc.vector.tensor_sub(num[:], num[:], snts[:])
            expnum = pp.tile([128, NB], F32)
            nc.scalar.activation(expnum[:], num[:], AF.Exp)

            # ---- denominator chain (tail) ----
            denom = pp.tile([128, NB], F32)
            nc.vector.tensor_add(denom[:], expnum[:], fullsum[:])
            nc.vector.tensor_sub(denom[:], denom[:], exptgt[:])
            logd = pp.tile([128, NB], F32)
            nc.scalar.activation(logd[:], denom[:], AF.Ln)
            lvals = pp.tile([128, NB], F32)
            nc.vector.tensor_sub(lvals[:], num[:], logd[:])
            lred = pp.tile([128, 1], F32)
            nc.vector.reduce_sum(lred[:], lvals[:], axis=mybir.AxisListType.X)
            zf = pmain.tile([128, 2048], F32, tag="z", name="zf")
            nc.tensor.matmul(zf[0:1, 0:1], ones_f32[:], lred[:],
                             start=True, stop=True)
            outv = pp.tile([1, 1], F32)
            nc.scalar.mul(outv[:], zf[0:1, 0:1], -1.0 / float(B))
            nc.sync.dma_start(out_ext[:], outv[:])
            pmain_cm.__exit__(None, None, None)

    nc.compile()
    return nc


def _prep_inputs(features, y_true, weight):
    features = np.asarray(features, dtype=np.float32)
    weight = np.asarray(weight, dtype=np.float32)
    y = np.asarray(y_true).astype(np.int64)

    # fT column tau*128+pi holds batch row pi*16+tau, matching the (p t)
    # SBUF mapping of fnat/wtgt (pure index permutation).
    perm = np.arange(B).reshape(128, NB).T.ravel()
    fT = features[perm].T.astype(FP8NP, order="C")     # [D, B]
    fnat = features.astype(BF16NP)                     # [B, D] bf16
    wtgt = weight[y].astype(BF16NP)                    # [B, D] bf16

    in_maps = []
    for i in range(NCORES):
        shard = weight[i * CS:(i + 1) * CS]            # [CS, D]
        wT = shard.T.astype(BF16NP, order="C")         # [D, CS]
        in_maps.append({"fT": fT, "wT": wT, "fnat": fnat, "wtgt": wtgt})
    return in_maps


def _run(features, y_true, weight, trace=False, **run_kwargs):
    if "nc" not in _CACHE:
        _CACHE["nc"] = _build()
    nc = _CACHE["nc"]
    in_maps = _prep_inputs(features, y_true, weight)
    res = run_bass_kernel_spmd(
        nc, in_maps, core_ids=list(range(NCORES)), trace=trace, **run_kwargs)
    out = np.asarray(res.results[0]["out"], dtype=np.float32)
    return np.float32(out.reshape(-1)[0]), res


def kernel(features, y_true, weight):
    val, _ = _run(features, y_true, weight, trace=False)
    return np.asarray(val, dtype=np.float32)


# revision 23
# speedup vs baseline: 1.2189x; 1.2189x over previous
"""ArcFace (AngularPenaltySMLoss) on 8 TRN2 NeuronCores.

Sharding (model-parallel softmax): 32768 classes split across 8 cores
(4096 each). Host prep is layout/dtype only (transpose, gather, dtype
casts, index permutation). No host arithmetic.

v4 structure:
  - DMA: big-contiguous-row transfers split across both HWDGE queues
    (sync + scalar) in dependency-priority order; fnat/wtgt use a
    (p t) partition mapping (16KB contiguous per partition) and fT
    ships with columns permuted host-side to match (pure indexing:
    column tau*128+pi holds batch row pi*16+tau).
  - Weight-col norms group 0 upfront: DVE squares of full k-rows,
    PE matmuls with a ones[128,128] stationary leave the column sums
    replicated across partitions; ACT exp(-0.5*ln) -> rnr; per-(chunk,
    k-pair) DVE muls quantize to fp8 whats. Group 1's norm matmuls are
    HOSTED inside sweep-A tiles b8..b11 (each 512-chunk's rowsums live
    in that tile's PSUM before the mains start-clear it), so no PSUM
    slot is held across the sweep and the head stays short.
  - Main loop class-half-major, j-outer (stationary reuse), fp8
    DoubleRow; ACT Exp (scale=64/||f||) with accum_out row sums.
  - ssf/rawdot/wn2 on Pool (muls) + DVE (reduces), ssf first.
  - ONE AllGather of the locally pre-added sums at the very end:
    serial per-collective turnaround on the collective cores plus the
    cross-core pipeline offset made multiple AllGathers strictly
    worse; everything except the last denominator chain is
    precomputed during the sweep.
"""
import math

import numpy as np
import ml_dtypes

import concourse.bass as bass
import concourse.tile as tile
from concourse import bacc, mybir
from concourse.bass_utils import run_bass_kernel_spmd

B = 2048          # batch
D = 512           # feature dim
C = 32768         # classes
NCORES = 8
CS = C // NCORES  # 4096 classes per core
S = 64.0
MARGIN = 0.5
EPS = 1e-7
COSM = math.cos(MARGIN)
SINM = math.sin(MARGIN)

NB = B // 128     # 16 batch tiles
NK = D // 128     # 4 contraction chunks
NCC = CS // 512   # 8 class chunks per core

F32 = mybir.dt.float32
BF16 = mybir.dt.bfloat16
AF = mybir.ActivationFunctionType
ALU = mybir.AluOpType
BF16NP = ml_dtypes.bfloat16
FP8 = mybir.dt.float8e4
FP8NP = ml_dtypes.float8_e4m3fn
DR = mybir.MatmulPerfMode.DoubleRow

_CACHE = {}

_ONE_SET = "natural_log_exp_and_others"


def _patch_act_tables():
    from concourse import hw_specs, bacc as bacc_mod
    if getattr(bacc_mod, "_act_tables_patched", False):
        return
    orig = hw_specs.get_activation_tables

    def patched(arch):
        t = orig(arch)
        return {name: (funcs if name == _ONE_SET else set())
                for name, funcs in t.items()}

    bacc_mod.get_activation_tables = patched
    bacc_mod._act_tables_patched = True


def _build():
    _patch_act_tables()
    nc = bacc.Bacc(None, target_bir_lowering=False, debug=False)

    fT_ext = nc.declare_dram_parameter("fT", [D, B], FP8, isOutput=False)
    wT_ext = nc.declare_dram_parameter("wT", [D, CS], BF16, isOutput=False)
    fnat_ext = nc.declare_dram_parameter("fnat", [B, D], BF16, isOutput=False)
    wtgt_ext = nc.declare_dram_parameter("wtgt", [B, D], BF16, isOutput=False)
    out_ext = nc.declare_dram_parameter("out", [1, 1], F32, isOutput=True)

    ccT_in = nc.dram_tensor("ccT_in", [128, NB], F32)
    ccT_out = nc.dram_tensor("ccT_out", [128 * NCORES, NB], F32,
                             addr_space="Shared")
    ccW_in = nc.dram_tensor("ccW_in", [128, 1], F32)
    ccW_out = nc.dram_tensor("ccW_out", [128 * NCORES, 1], F32,
                             addr_space="Shared")

    with tile.TileContext(nc) as tc:
        with (
            tc.tile_pool(name="persist", bufs=1) as pp,
            tc.tile_pool(name="stream", bufs=4) as sp,
        ):
            # ---- persistent SBUF tiles ----
            wt3 = pp.tile([128, NK, CS], BF16)     # raw wT (bf16)
            whats = [pp.tile([128, NK, 512], FP8, tag=f"what{i}",
                             name=f"what{i}")
                     for i in range(NCC)]          # normalized wT, per chunk
            ft3 = pp.tile([128, NK, B], FP8)       # raw fT (fp8) = stationary
            fnat3 = pp.tile([128, NB, D], BF16)    # features, (p t) mapping
            wtgt3 = pp.tile([128, NB, D], BF16)    # target rows, (p t)
            ones128 = pp.tile([128, 128], BF16)
            ones_f32 = pp.tile([128, 1], F32)
            rnr_sb = pp.tile([128, NCC, 512], BF16)  # 1/||w_c||, replicated
            lnr_sb = pp.tile([128, 2048], F32)       # ln scratch
            ejunk = pp.tile([128, 2048], BF16)       # Exp write target
            sumsA = pp.tile([128, NB], F32)        # exp sums, chunks 0-3
            sumsB = pp.tile([128, NB], F32)        # exp sums, chunks 4-7
            rs_pt = pp.tile([128, NB], F32)        # 64/||f_b|| per-partition
            ssf = pp.tile([128, NB], F32)
            rawdot = pp.tile([128, NB], F32)
            wn2 = pp.tile([128, NB], F32)
            sq1 = [pp.tile([128, 2048], BF16, tag=f"sq{k}", name=f"sq{k}")
                   for k in range(NK)]             # squares of g1 k-rows

            # ---- DMAs: big rows, two hwdge queues, priority order ----
            wTr = wT_ext[:].rearrange("(k p) c -> p k c", p=128)
            fTr = fT_ext[:].rearrange("(k p) b -> p k b", p=128)
            fnr = fnat_ext[:].rearrange("(p t) d -> p t d", t=NB)
            wgr = wtgt_ext[:].rearrange("(p t) d -> p t d", t=NB)
            # ~2KB descriptor runs: fine enough that the 8 cores'
            # simultaneous demand round-robins fairly at the HBM (large
            # packets let the first-launched core win arbitration and
            # pushed the laggard ~40us late into the final AllGather)
            g0 = slice(0, 2048)
            g1 = slice(2048, 4096)
            for h in range(2):
                q = slice(1024 * h, 1024 * (h + 1))
                nc.sync.dma_start(wt3[:, 0, q], wTr[:, 0, q])
                nc.scalar.dma_start(wt3[:, 2, q], wTr[:, 2, q])
                nc.sync.dma_start(wt3[:, 1, q], wTr[:, 1, q])
                nc.scalar.dma_start(wt3[:, 3, q], wTr[:, 3, q])
            for k in range(NK):
                eng = nc.sync if k % 2 == 0 else nc.scalar
                eng.dma_start(ft3[:, k, :], fTr[:, k, :])
            for t0 in range(0, 8, 2):
                nc.scalar.dma_start(fnat3[:, t0:t0 + 2, :],
                                    fnr[:, t0:t0 + 2, :])
            for h in range(2, 4):
                q = slice(1024 * h, 1024 * (h + 1))
                nc.sync.dma_start(wt3[:, 0, q], wTr[:, 0, q])
                nc.scalar.dma_start(wt3[:, 2, q], wTr[:, 2, q])
                nc.sync.dma_start(wt3[:, 1, q], wTr[:, 1, q])
                nc.scalar.dma_start(wt3[:, 3, q], wTr[:, 3, q])
            for t0 in range(0, NB, 2):
                nc.sync.dma_start(wtgt3[:, t0:t0 + 2, :],
                                  wgr[:, t0:t0 + 2, :])
            for t0 in range(8, NB, 2):
                nc.scalar.dma_start(fnat3[:, t0:t0 + 2, :],
                                    fnr[:, t0:t0 + 2, :])

            nc.vector.memset(ones128[:], 1.0)
            nc.vector.memset(ones_f32[:], 1.0)

            # dummy AllGather: warms the collective mesh and absorbs the
            # cross-core launch skew in parallel with the compute, so the
            # real AllGather at the tail sees aligned peers and a formed
            # mesh. Nothing reads ccW_out.
            nc.sync.dma_start(ccW_in[:], ones_f32[:])
            nc.gpsimd.collective_compute(
                "AllGather", ALU.bypass,
                replica_groups=[list(range(NCORES))],
                ins=[ccW_in[:].opt()],
                outs=[ccW_out[:].opt()],
            )

            pmain_cm = tc.tile_pool(name="pmain", bufs=2, space="PSUM")
            pmain = pmain_cm.__enter__()

            # ---- group-0 weight-column norms (upfront) ----
            npsA = pmain.tile([128, 2048], F32, tag="z", name="npsA")
            for k in range(NK):
                sqk = sp.tile([128, 2048], BF16, tag="sqt", name="sqk")
                nc.vector.tensor_mul(sqk[:], wt3[:, k, g0], wt3[:, k, g0])
                for c4 in range(4):
                    nc.tensor.matmul(
                        npsA[:, bass.ts(c4, 512)],
                        ones128[:], sqk[:, bass.ts(c4, 512)],
                        start=(k == 0), stop=(k == NK - 1))
            for h in range(2):
                seg = npsA[:, h * 1024:(h + 1) * 1024]
                lseg = lnr_sb[:, h * 1024:(h + 1) * 1024]
                nc.scalar.activation(lseg, seg, AF.Ln)
                nc.scalar.activation(
                    rnr_sb[:, 2 * h:2 * h + 2, :]
                    .rearrange("p a b -> p (a b)"),
                    lseg, AF.Exp, scale=-0.5)

            # whats g0, j0 first (mains b0-j0 needs only the k0-k1 pair)
            def what_mul(cc, j):
                nc.vector.tensor_mul(
                    whats[cc][:, 2 * j:2 * j + 2, :],
                    wt3[:, 2 * j:2 * j + 2, bass.ts(cc, 512)],
                    rnr_sb[:, cc:cc + 1, :].broadcast_to([128, 2, 512]))

            for cc in range(4):
                what_mul(cc, 0)
            for cc in range(4):
                what_mul(cc, 1)
            # squares of the g1 k-rows: after whats-g0 (so mains are not
            # blocked) but before the Pool-paced reduces (so the hosted
            # sweep-A norm matmuls are never input-starved)
            for k in range(NK):
                nc.vector.tensor_mul(sq1[k][:], wt3[:, k, g1], wt3[:, k, g1])

            # ---- Pool muls + DVE reduces: ssf first ----
            for t in range(NB):
                sqf = sp.tile([128, D], BF16, tag="prod", name="sqf")
                nc.gpsimd.tensor_mul(sqf[:], fnat3[:, t, :], fnat3[:, t, :])
                nc.vector.reduce_sum(ssf[:, t:t + 1], sqf[:],
                                     axis=mybir.AxisListType.X)
            for t in range(NB):
                prod = sp.tile([128, D], BF16, tag="prod", name="prod")
                nc.gpsimd.tensor_mul(prod[:], fnat3[:, t, :], wtgt3[:, t, :])
                nc.vector.reduce_sum(rawdot[:, t:t + 1], prod[:],
                                     axis=mybir.AxisListType.X)
                sq2 = sp.tile([128, D], BF16, tag="prod", name="sq2")
                nc.gpsimd.tensor_mul(sq2[:], wtgt3[:, t, :], wtgt3[:, t, :])
                nc.vector.reduce_sum(wn2[:, t:t + 1], sq2[:],
                                     axis=mybir.AxisListType.X)

            # rs = 64/||f|| = exp(-0.5*ln(ssf/4096)); small first batches
            for h0, h1 in ((0, 2), (2, 4), (4, 8), (8, 12), (12, 16)):
                lcol = sp.tile([128, 4], F32, tag="lcol", name="lcol")
                nc.scalar.activation(lcol[:, 0:h1 - h0], ssf[:, h0:h1],
                                     AF.Ln, scale=1.0 / 4096.0)
                nc.scalar.activation(rs_pt[:, h0:h1], lcol[:, 0:h1 - h0],
                                     AF.Exp, scale=-0.5)

            # ---- main sweeps ----
            def mains(zp, g, b):
                for j in range(2):
                    for c4 in range(4):
                        cc = 4 * g + c4
                        nc.tensor.matmul(
                            zp[:, bass.ts(c4, 512)],
                            ft3[:, 2 * j:2 * j + 2, bass.ts(b, 128)],
                            whats[cc][:, 2 * j:2 * j + 2, :],
                            start=(j == 0), stop=(j == 1),
                            perf_mode=DR)

            # sweep A; tiles b8..b11 host the group-1 norm rowsums in
            # their PSUM pre-life (region is start-cleared by the mains
            # only after the Ln has read it)
            for b in range(NB):
                zp = pmain.tile([128, 2048], F32, tag="z", name="zp")
                if 8 <= b < 12:
                    cc = b - 4          # chunks 4..7
                    c4 = b - 8
                    for k in range(NK):
                        nc.tensor.matmul(
                            zp[:, 0:512], ones128[:],
                            sq1[k][:, bass.ts(c4, 512)],
                            start=(k == 0), stop=(k == NK - 1))
                    nc.scalar.activation(lnr_sb[:, 0:512], zp[:, 0:512],
                                         AF.Ln)
                    nc.scalar.activation(rnr_sb[:, cc, :], lnr_sb[:, 0:512],
                                         AF.Exp, scale=-0.5)
                    what_mul(cc, 0)
                    what_mul(cc, 1)
                mains(zp, 0, b)
                nc.scalar.activation(
                    ejunk[:], zp[:], AF.Exp, scale=rs_pt[:, b:b + 1],
                    accum_out=sumsA[:, b:b + 1])
            # sweep B
            for b in range(NB):
                zp = pmain.tile([128, 2048], F32, tag="z", name="zpB")
                mains(zp, 1, b)
                nc.scalar.activation(
                    ejunk[:], zp[:], AF.Exp, scale=rs_pt[:, b:b + 1],
                    accum_out=sumsB[:, b:b + 1])

            # ---- single AllGather of the pre-added sums ----
            sumsT = pp.tile([128, NB], F32)
            nc.vector.tensor_add(sumsT[:], sumsA[:], sumsB[:])
            nc.sync.dma_start(ccT_in[:], sumsT[:])
            nc.gpsimd.collective_compute(
                "AllGather", ALU.bypass,
                replica_groups=[list(range(NCORES))],
                ins=[ccT_in[:].opt()],
                outs=[ccT_out[:].opt()],
            )
            gathT = pp.tile([128, NCORES, NB], F32)
            nc.sync.dma_start(
                gathT[:], ccT_out[:].rearrange("(g p) c -> p g c", p=128))
            fullsum = pp.tile([128, NB], F32)
            nc.vector.tensor_reduce(
                fullsum[:], gathT[:].rearrange("p g c -> p c g"),
                axis=mybir.AxisListType.X, op=ALU.add)

            # ---- combine: everything except the denominator is
            # precomputed during the sweep ----
            m2 = pp.tile([128, NB], F32)
            nc.vector.tensor_mul(m2[:], ssf[:], wn2[:])
            lm2 = pp.tile([128, NB], F32)
            nc.scalar.activation(lm2[:], m2[:], AF.Ln)
            rboth = pp.tile([128, NB], F32)
            nc.scalar.activation(rboth[:], lm2[:], AF.Exp, scale=-0.5)
            tgt = pp.tile([128, NB], F32)
            nc.vector.tensor_mul(tgt[:], rawdot[:], rboth[:])
            exptgt = pp.tile([128, NB], F32)
            nc.scalar.activation(exptgt[:], tgt[:], AF.Exp, scale=S)
            tclip = pp.tile([128, NB], F32)
            nc.vector.tensor_scalar(
                tclip[:], tgt[:], -1.0 + EPS, 1.0 - EPS,
                op0=ALU.max, op1=ALU.min)
            om = pp.tile([128, NB], F32)
            nc.vector.tensor_mul(om[:], tclip[:], tclip[:])
            nc.vector.tensor_scalar(om[:], om[:], -1.0, 1.0,
                                    op0=ALU.mult, op1=ALU.add)
            lom = pp.tile([128, NB], F32)
            nc.scalar.activation(lom[:], om[:], AF.Ln)
            snt = pp.tile([128, NB], F32)
            nc.scalar.activation(snt[:], lom[:], AF.Exp, scale=0.5)
            num = pp.tile([128, NB], F32)
            nc.vector.tensor_scalar_mul(num[:], tclip[:], S * COSM)
            snts = pp.tile([128, NB], F32)
            nc.vector.tensor_scalar_mul(snts[:], snt[:], S * SINM)
            nc.vector.tensor_sub(num[:], num[:], snts[:])
            expnum = pp.tile([128, NB], F32)
            nc.scalar.activation(expnum[:], num[:], AF.Exp)

            # ---- denominator chain (tail) ----
            denom = pp.tile([128, NB], F32)
            nc.vector.tensor_add(denom[:], expnum[:], fullsum[:])
            nc.vector.tensor_sub(denom[:], denom[:], exptgt[:])
            logd = pp.tile([128, NB], F32)
            nc.scalar.activation(logd[:], denom[:], AF.Ln)
            lvals = pp.tile([128, NB], F32)
            nc.vector.tensor_sub(lvals[:], num[:], logd[:])
            lred = pp.tile([128, 1], F32)
            nc.vector.reduce_sum(lred[:], lvals[:], axis=mybir.AxisListType.X)
            zf = pmain.tile([128, 2048], F32, tag="z", name="zf")
            nc.tensor.matmul(zf[0:1, 0:1], ones_f32[:], lred[:],
                             start=True, stop=True)
            outv = pp.tile([1, 1], F32)
            nc.scalar.mul(outv[:], zf[0:1, 0:1], -1.0 / float(B))
            nc.sync.dma_start(out_ext[:], outv[:])
            pmain_cm.__exit__(None, None, None)

    nc.compile()
    return nc


def _prep_inputs(features, y_true, weight):
    features = np.asarray(features, dtype=np.float32)
    weight = np.asarray(weight, dtype=np.float32)
    y = np.asarray(y_true).astype(np.int64)

    # fT column tau*128+pi holds batch row pi*16+tau, matching the (p t)
    # SBUF mapping of fnat/wtgt (pure index permutation).
    perm = np.arange(B).reshape(128, NB).T.ravel()
    fT = features[perm].T.astype(FP8NP, order="C")     # [D, B]
    fnat = features.astype(BF16NP)                     # [B, D] bf16
    wtgt = weight[y].astype(BF16NP)                    # [B, D] bf16

    in_maps = []
    for i in range(NCORES):
        shard = weight[i * CS:(i + 1) * CS]            # [CS, D]
        wT = shard.T.astype(BF16NP, order="C")         # [D, CS]
        in_maps.append({"fT": fT, "wT": wT, "fnat": fnat, "wtgt": wtgt})
    return in_maps


def _run(features, y_true, weight, trace=False, **run_kwargs):
    if "nc" not in _CACHE:
        _CACHE["nc"] = _build()
    nc = _CACHE["nc"]
    in_maps = _prep_inputs(features, y_true, weight)
    res = run_bass_kernel_spmd(
        nc, in_maps, core_ids=list(range(NCORES)), trace=trace, **run_kwargs)
    out = np.asarray(res.results[0]["out"], dtype=np.float32)
    return np.float32(out.reshape(-1)[0]), res


def kernel(features, y_true, weight):
    val, _ = _run(features, y_true, weight, trace=False)
    return np.asarray(val, dtype=np.float32)
